# revision 1
# baseline (speedup 1.0000x reference)
"""CaNet GAT-style layer on 8 Trainium2 NeuronCores.

Algorithm (matches the jax reference):
  h[k]   = x @ W[k]                      per-head projection
  s_src  = h[k] @ a_src[k],  s_dst = h[k] @ a_dst[k]
  w_e    = exp(leakyrelu(s_src[src] + s_dst[dst]))       (softmax shift
           invariance makes the reference's global-max subtraction a no-op
           up to the +1e-8 epsilon; exp args here are bounded ~|u|<=15)
  hi[k,i] = sum_{e: src[e]=i} w_e * h[k, dst[e]]
  den[k,i]= sum_{e: src[e]=i} w_e + 1e-8
  out[i] = sum_k e[i,k] * hi[k,i]/den[k,i] + x[i]

Distribution: nodes are split into 8 contiguous shards (one per core);
edges are partitioned by their segment id (src).  Each core redundantly
computes the full node table T (h in bf16 + per-node s_src/s_dst in f32,
plus interleaved 1.0 columns so one elementwise multiply produces both the
scaled messages and the softmax denominator columns), then gathers rows of
T by dst via indirect DMA (on HW each indirect_dma_start consumes one
index per partition; keep one uniform stream of full-row gathers — tiny
or interleaved small calls destroy SWDGE pipelining), and does the
per-segment reduction with one-hot matrices (built on the host from adj
only) on the tensor engine, 2 heads per matmul, accumulating in PSUM per
128-node block.  Per-edge s_src comes from one row-gather per 128-node
block expanded through the transposed one-hot on the PE (transpose + ACT
copy + tiny matmul), not a per-edge gather.  Measured ~0.77 ms device
time for the full inference (8 cores), absmax-relative error vs the fp32
reference ~9.4e-4 (bf16 message table).
"""

import math
import os
import sys
from contextlib import ExitStack

import numpy as np

for _p in ("/opt/trn_rl_repo", "/root/.axon_site/_ro/trn_rl_repo"):
    if os.path.isdir(_p) and _p not in sys.path:
        sys.path.insert(0, _p)

import ml_dtypes

import concourse.bass as bass
import concourse.mybir as mybir
import concourse.tile as tile
from concourse import bacc
from concourse.bass_utils import run_bass_kernel_spmd
from concourse.masks import make_identity

# problem constants (hardcoded per contract)
N = 50000
F = 128
K = 4
NCORES = 8
NPC = N // NCORES          # 6250 nodes per core
BLK = 128                  # segment block = psum partition dim
NBLK = math.ceil(NPC / BLK)  # 49 blocks per core (last covers 106 nodes)
NPAD = NBLK * BLK          # 6272
M_SUB = 8                  # sub-tiles (128 edges each) per gather call
GRP = 130                  # bf16 cols per head group: 128 h + 1.0 + pad
ROW = K * GRP + 16         # 536 bf16: 4 head groups + 4xf32 s_src + 4xf32 s_dst
SSRC_OFF = K * GRP         # 520 (bf16 units)
NCHUNK = math.ceil(N / 128)  # 391 node chunks in phase A
TROWS = NCHUNK * 128       # 50048 table rows (padded)
ALPHA = 0.01
EPS = 1e-8

f32 = mybir.dt.float32
bf16 = mybir.dt.bfloat16
i32 = mybir.dt.int32

_CACHE = {}
# experiment knobs (default = shipping config)
_OPTS = {"ssrc_gather": True, "big_gather": True, "nqueues": 1,
         "ssrc_mode": "fullrow"}  # "aligned" | "offset16" | "fullrow"
SSG_OFF = 512   # aligned small-gather start (bf16 units; 1024B-aligned)
SSG_LEN = 64    # 128B per row


def _prep(x, e, weights, a, adj):
    """Host-side sharding/indexing prep. Only touches adj for structure;
    all floating point math happens on device."""
    Et = adj.shape[1] + N
    src = np.concatenate([adj[0], np.arange(N, dtype=adj.dtype)]).astype(np.int64)
    dst = np.concatenate([adj[1], np.arange(N, dtype=adj.dtype)]).astype(np.int64)

    core = src // NPC
    rel = src - core * NPC
    blk = rel // BLK                    # < NBLK since rel < 6250
    srcrel = rel - blk * BLK
    key = (core * NBLK + blk).astype(np.int64)

    order = np.argsort(key, kind="stable")
    skey = key[order]
    ssrcrel = srcrel[order].astype(np.int64)
    sdst = dst[order]
    ssrc = src[order]

    counts = np.bincount(skey, minlength=NCORES * NBLK)
    TPB = int(math.ceil(counts.max() / 128))          # sub-tiles per block
    S_TOT = NBLK * TPB
    G = math.ceil(S_TOT / M_SUB)
    S_PAD = G * M_SUB

    starts = np.zeros(NCORES * NBLK + 1, dtype=np.int64)
    np.cumsum(counts, out=starts[1:])
    rank = np.arange(Et, dtype=np.int64) - starts[skey]

    c_of = skey // NBLK
    b_of = skey % NBLK
    sub = b_of * TPB + rank // 128
    slot = rank % 128
    g_of = sub // M_SUB
    m_of = sub % M_SUB

    A_all = np.zeros((NCORES, G, 128, M_SUB, 128), dtype=ml_dtypes.bfloat16)
    di_all = np.zeros((NCORES, G, 128, M_SUB), dtype=np.int32)
    si_all = np.zeros((NCORES, G, 128, M_SUB), dtype=np.int32)
    A_all[c_of, g_of, slot, m_of, ssrcrel] = 1.0
    di_all[c_of, g_of, slot, m_of] = sdst
    si_all[c_of, g_of, slot, m_of] = ssrc

    xT = np.zeros((128, TROWS), dtype=np.float32)
    xT[:, :N] = x.T
    xm = np.zeros((NCORES, NPAD, F), dtype=np.float32)
    em = np.zeros((NCORES, NPAD, K), dtype=np.float32)
    for c in range(NCORES):
        xm[c, :NPC] = x[c * NPC:(c + 1) * NPC]
        em[c, :NPC] = e[c * NPC:(c + 1) * NPC]

    # per-(core,block) global node ids: one table-row gather per block
    bi = np.zeros((NCORES, NPAD, 1), dtype=np.int32)
    for c in range(NCORES):
        ids = c * NPC + np.arange(NPAD)
        bi[c, :, 0] = np.minimum(ids, N - 1)

    meta = dict(TPB=TPB, G=G, S_PAD=S_PAD)
    in_maps = []
    for c in range(NCORES):
        in_maps.append({
            "xT": xT,
            "w": np.ascontiguousarray(weights.astype(np.float32)),
            "a": np.ascontiguousarray(a.astype(np.float32)),
            "Ah": np.ascontiguousarray(A_all[c].reshape(G, 128, M_SUB * 128)),
            "di": np.ascontiguousarray(di_all[c]),
            "si": np.ascontiguousarray(si_all[c]),
            "bi": np.ascontiguousarray(bi[c]),
            "xm": np.ascontiguousarray(xm[c]),
            "em": np.ascontiguousarray(em[c]),
        })
    return meta, in_maps


def _build(meta):
    TPB, G, S_PAD = meta["TPB"], meta["G"], meta["S_PAD"]
    nc = bacc.Bacc(num_swdge_queues=_OPTS["nqueues"])

    xT = nc.declare_dram_parameter("xT", [128, TROWS], f32, isOutput=False)
    W = nc.declare_dram_parameter("w", [K, 128, 128], f32, isOutput=False)
    Aa = nc.declare_dram_parameter("a", [K, 256, 1], f32, isOutput=False)
    Ah = nc.declare_dram_parameter("Ah", [G, 128, M_SUB * 128], bf16, isOutput=False)
    DI = nc.declare_dram_parameter("di", [G, 128, M_SUB], i32, isOutput=False)
    SI = nc.declare_dram_parameter("si", [G, 128, M_SUB], i32, isOutput=False)
    XM = nc.declare_dram_parameter("xm", [NPAD, F], f32, isOutput=False)
    EM = nc.declare_dram_parameter("em", [NPAD, K], f32, isOutput=False)
    BI = nc.declare_dram_parameter("bi", [NPAD, 1], i32, isOutput=False)
    OUT = nc.declare_dram_parameter("out", [NPAD, F], f32, isOutput=True)

    Copy = mybir.ActivationFunctionType.Copy
    Exp = mybir.ActivationFunctionType.Exp

    with ExitStack() as ctx:
        tc = ctx.enter_context(tile.TileContext(nc))
        const = ctx.enter_context(tc.tile_pool(name="const", bufs=1))
        sb = ctx.enter_context(tc.tile_pool(name="sb", bufs=3))
        sbG = ctx.enter_context(tc.tile_pool(name="sbG", bufs=3))
        dram = ctx.enter_context(tc.tile_pool(name="dram", bufs=1, space="DRAM"))

        # +128 pad rows: the aligned small gather reads a bit past row end
        Ttab = dram.tile([TROWS + 128, ROW], bf16, tag="Ttab")

        ident = const.tile([128, 128], f32, tag="ident")
        make_identity(nc, ident[:])
        onespad = const.tile([128, 2], bf16, tag="onespad")
        nc.gpsimd.memset(onespad[:, 0:1], 1.0)
        nc.gpsimd.memset(onespad[:, 1:2], 0.0)
        ident_bf = const.tile([128, 128], bf16, tag="ident_bf")
        nc.vector.tensor_copy(ident_bf[:], ident[:])

        # ---- phase A one-time: Wext[k] = [W_k | va_k | vb_k] ----
        wext = []
        with tc.tile_pool(name="psA1", bufs=1, space="PSUM") as psA1:
            for k in range(K):
                wk = const.tile([128, GRP], f32, tag=f"wext{k}")
                nc.sync.dma_start(out=wk[:, 0:128], in_=W[k])
                ak = sb.tile([128, 2], f32, tag="ak")
                nc.sync.dma_start(out=ak[:, 0:1], in_=Aa[k, 0:128, :])
                nc.sync.dma_start(out=ak[:, 1:2], in_=Aa[k, 128:256, :])
                pT = psA1.tile([128, 128], f32, tag="pT")
                nc.tensor.transpose(pT[:], wk[:, 0:128], ident[:])
                wkT = sb.tile([128, 128], f32, tag="wkT")
                nc.vector.tensor_copy(wkT[:], pT[:])
                pva = psA1.tile([128, 2], f32, tag="pva")
                nc.tensor.matmul(pva[:], wkT[:], ak[:], start=True, stop=True)
                nc.vector.tensor_copy(wk[:, 128:130], pva[:])
                wext.append(wk)

        # ---- phase A: build node table T ----
        with tc.tile_pool(name="psA", bufs=2, space="PSUM") as psA:
            for c in range(NCHUNK):
                xc = sb.tile([128, 128], f32, tag="xc")
                nc.sync.dma_start(out=xc[:], in_=xT[:, c * 128:(c + 1) * 128])
                p01 = psA.tile([128, 2 * GRP], f32, tag="p01")
                p23 = psA.tile([128, 2 * GRP], f32, tag="p23")
                for k in range(K):
                    pt = p01 if k < 2 else p23
                    j = (k % 2) * GRP
                    nc.tensor.matmul(pt[:, j:j + GRP], xc[:], wext[k][:],
                                     start=True, stop=True)
                tsb = sb.tile([128, ROW], bf16, tag="tsb")
                p01v = p01[:].rearrange("p (k o) -> p k o", o=GRP)
                p23v = p23[:].rearrange("p (k o) -> p k o", o=GRP)
                t01 = tsb[:, 0:2 * GRP].rearrange("p (k o) -> p k o", o=GRP)
                t23 = tsb[:, 2 * GRP:4 * GRP].rearrange("p (k o) -> p k o", o=GRP)
                # h columns (f32 psum -> bf16), split across ACT and DVE
                nc.scalar.copy(out=t01[:, :, 0:128], in_=p01v[:, :, 0:128])
                nc.vector.tensor_copy(out=t23[:, :, 0:128], in_=p23v[:, :, 0:128])
                # the interleaved [1.0, 0.0] columns
                allv = tsb[:, 0:4 * GRP].rearrange("p (k o) -> p k o", o=GRP)
                nc.vector.tensor_copy(
                    out=allv[:, :, 128:130],
                    in_=onespad[:, None, :].to_broadcast([128, 4, 2]))
                # s_src (psum col 128) and s_dst (psum col 129) f32 fields
                tf = tsb[:, SSRC_OFF:SSRC_OFF + 16].bitcast(f32)  # [128, 8]
                tfv = tf.rearrange("p (f k) -> p k f", f=2)  # [p, k:2, f:2] view
                # per psum pair: out cols {k, 4+k}  <- psum cols {128, 129}
                nc.vector.tensor_copy(out=tfv[:, 0:2, :], in_=p01v[:, :, 128:130])
                nc.scalar.copy(out=tfv[:, 2:4, :], in_=p23v[:, :, 128:130])
                nc.sync.dma_start(out=Ttab[c * 128:(c + 1) * 128, :], in_=tsb[:])

        # ---- gather + segment reduction ----
        with tc.tile_pool(name="psHI", bufs=2, space="PSUM") as psHI, \
                tc.tile_pool(name="psG", bufs=2, space="PSUM") as psG:
            curA = curB = None
            cur_ssb = None
            for g in range(G):
                asb = sbG.tile([128, M_SUB * 128], bf16, tag="asb")
                nc.sync.dma_start(out=asb[:], in_=Ah[g])
                dib = sbG.tile([128, M_SUB], i32, tag="dib")
                nc.sync.dma_start(out=dib[:], in_=DI[g])

                graw = sbG.tile([128, M_SUB * ROW], bf16, tag="graw")
                for m in range(M_SUB):
                    # HW indirect DMA: one index per partition per call
                    nc.gpsimd.indirect_dma_start(
                        out=graw[:, m * ROW:(m + 1) * ROW], out_offset=None,
                        in_=Ttab[:, :],
                        in_offset=bass.IndirectOffsetOnAxis(
                            ap=dib[:, m:m + 1], axis=0))

                # per-edge s_src: one table-row gather per 128-node block,
                # expanded per edge with the transposed one-hot on the PE
                psU = psG.tile([128, M_SUB * K], f32, tag="psU")
                for m in range(M_SUB):
                    s = g * M_SUB + m
                    b = min(s // TPB, NBLK - 1)
                    if s == b * TPB:
                        bib = sbG.tile([128, 1], i32, tag="bib")
                        nc.sync.dma_start(out=bib[:],
                                          in_=BI[b * 128:(b + 1) * 128, :])
                        rowb = sbG.tile([128, ROW], bf16, tag="rowb")
                        nc.gpsimd.indirect_dma_start(
                            out=rowb[:], out_offset=None, in_=Ttab[:, :],
                            in_offset=bass.IndirectOffsetOnAxis(
                                ap=bib[:], axis=0))
                        ssbt = sbG.tile([128, K], f32, tag="ssbt")
                        nc.vector.tensor_copy(
                            out=ssbt[:],
                            in_=rowb[:, SSRC_OFF:SSRC_OFF + 16].bitcast(f32)[:, 0:4])
                        cur_ssb = ssbt
                    pT2 = psG.tile([128, 128], bf16, tag="pT2")
                    nc.tensor.transpose(pT2[:], asb[:, m * 128:(m + 1) * 128],
                                        ident_bf[:])
                    atm = sbG.tile([128, 128], f32, tag="atm")
                    nc.scalar.copy(out=atm[:], in_=pT2[:])
                    nc.tensor.matmul(psU[:, m * K:(m + 1) * K], atm[:],
                                     cur_ssb[:], start=True, stop=True)

                # u = s_src[src] + s_dst[dst]; w = exp(leakyrelu(u)) in bf16
                grawv = graw[:].rearrange("p (m r) -> p m r", r=ROW)
                sdv = grawv[:, :, SSRC_OFF:SSRC_OFF + 16].bitcast(f32) \
                    .rearrange("p m (f k) -> p m f k", f=2)
                uv = sbG.tile([128, M_SUB * K], f32, tag="uv")
                uvv = uv[:].rearrange("p (m k) -> p m k", k=K)
                nc.vector.tensor_tensor(
                    out=uvv,
                    in0=psU[:].rearrange("p (m k) -> p m k", k=K),
                    in1=sdv[:, :, 1, :],
                    op=mybir.AluOpType.add)
                tv = sbG.tile([128, M_SUB * K], f32, tag="tv")
                nc.vector.tensor_scalar_mul(tv[:], uv[:], ALPHA)
                lv = sbG.tile([128, M_SUB * K], f32, tag="lv")
                nc.vector.tensor_max(lv[:], uv[:], tv[:])
                wb = sbG.tile([128, M_SUB * K], bf16, tag="wb")
                nc.scalar.activation(wb[:], lv[:], Exp)

                # Gs = w * [h_k | 1 | 0]  (both message and denominator cols)
                gs = sbG.tile([128, M_SUB * K * GRP], bf16, tag="gs")
                gsv = gs[:].rearrange("p (m k o) -> p m k o", k=K, o=GRP)
                ghv = grawv[:, :, 0:K * GRP].rearrange("p m (k o) -> p m k o", o=GRP)
                wbv = wb[:].rearrange("p (m k) -> p m k", k=K)[:, :, :, None] \
                    .to_broadcast([128, M_SUB, K, GRP])
                nc.vector.tensor_tensor(out=gsv, in0=ghv, in1=wbv,
                                        op=mybir.AluOpType.mult)

                for m in range(M_SUB):
                    s = g * M_SUB + m
                    b = min(s // TPB, NBLK - 1)
                    first = (s == b * TPB)
                    if b < NBLK - 1:
                        last = (s == b * TPB + TPB - 1)
                    else:
                        last = (s == S_PAD - 1)
                    if first:
                        curA = psHI.tile([128, 2 * GRP], f32, tag="hiA")
                        curB = psHI.tile([128, 2 * GRP], f32, tag="hiB")
                    lhsT = asb[:, m * 128:(m + 1) * 128]
                    o = m * K * GRP
                    nc.tensor.matmul(curA[:], lhsT, gs[:, o:o + 2 * GRP],
                                     start=first, stop=last)
                    nc.tensor.matmul(curB[:], lhsT, gs[:, o + 2 * GRP:o + 4 * GRP],
                                     start=first, stop=last)
                    if last:
                        _finalize(nc, sb, b, curA, curB, XM, EM, OUT, Copy)
    nc.finalize()
    return nc


def _finalize(nc, sb, b, hA, hB, XM, EM, OUT, Copy):
    r0, r1 = b * 128, (b + 1) * 128
    xb = sb.tile([128, F], f32, tag="xb")
    nc.sync.dma_start(out=xb[:], in_=XM[r0:r1, :])
    eb = sb.tile([128, K], f32, tag="eb")
    nc.sync.dma_start(out=eb[:], in_=EM[r0:r1, :])
    hAv = hA[:].rearrange("p (k o) -> p k o", o=GRP)
    hBv = hB[:].rearrange("p (k o) -> p k o", o=GRP)
    d4 = sb.tile([128, K], f32, tag="d4")
    nc.vector.tensor_scalar_add(d4[:, 0:2], hAv[:, :, 128], EPS)
    nc.vector.tensor_scalar_add(d4[:, 2:4], hBv[:, :, 128], EPS)
    r4 = sb.tile([128, K], f32, tag="r4")
    nc.vector.reciprocal(r4[:], d4[:])
    s4 = sb.tile([128, K], f32, tag="s4")
    nc.vector.tensor_mul(s4[:], r4[:], eb[:])
    t_ = [sb.tile([128, F], f32, tag=f"t{k}", name=f"t{k}") for k in range(K)]
    nc.vector.tensor_scalar_mul(t_[0][:], hAv[:, 0, 0:128], s4[:, 0:1])
    nc.scalar.activation(t_[1][:], hAv[:, 1, 0:128], Copy, scale=s4[:, 1:2])
    nc.vector.tensor_scalar_mul(t_[2][:], hBv[:, 0, 0:128], s4[:, 2:3])
    nc.scalar.activation(t_[3][:], hBv[:, 1, 0:128], Copy, scale=s4[:, 3:4])
    q0 = sb.tile([128, F], f32, tag="q0")
    nc.vector.tensor_add(q0[:], t_[0][:], t_[1][:])
    q1 = sb.tile([128, F], f32, tag="q1")
    nc.vector.tensor_add(q1[:], t_[2][:], t_[3][:])
    q2 = sb.tile([128, F], f32, tag="q2")
    nc.vector.tensor_add(q2[:], q0[:], q1[:])
    acc = sb.tile([128, F], f32, tag="acc")
    nc.vector.tensor_add(acc[:], q2[:], xb[:])
    nc.sync.dma_start(out=OUT[r0:r1, :], in_=acc[:])


def kernel(x, e, weights, a, adj):
    meta, in_maps = _prep(np.asarray(x), np.asarray(e), np.asarray(weights),
                          np.asarray(a), np.asarray(adj))
    ck = (meta["TPB"], meta["G"])
    if ck not in _CACHE:
        _CACHE[ck] = _build(meta)
    nc = _CACHE[ck]
    res = run_bass_kernel_spmd(nc, in_maps, list(range(NCORES)))
    out = np.empty((N, F), dtype=np.float32)
    for c in range(NCORES):
        out[c * NPC:(c + 1) * NPC] = res.results[c]["out"][:NPC]
    return out



# revision 4
# speedup vs baseline: 565.5690x; 565.5690x over previous
"""CaNet GAT-style layer on 8 Trainium2 NeuronCores.

Algorithm (matches the jax reference):
  h[k]   = x @ W[k]                      per-head projection
  s_src  = h[k] @ a_src[k],  s_dst = h[k] @ a_dst[k]
  w_e    = exp(leakyrelu(s_src[src] + s_dst[dst]))       (softmax shift
           invariance makes the reference's global-max subtraction a no-op
           up to the +1e-8 epsilon; exp args here are bounded ~|u|<=15)
  hi[k,i] = sum_{e: src[e]=i} w_e * h[k, dst[e]]
  den[k,i]= sum_{e: src[e]=i} w_e + 1e-8
  out[i] = sum_k e[i,k] * hi[k,i]/den[k,i] + x[i]

Distribution: nodes split into 8 contiguous shards (one per core); edges
partitioned by segment id (src).  Each core redundantly computes the full
node table Tmsg = [s_dst f32x4 | h fp8e4 4x128] (528 B/row), gathers rows
by dst via indirect DMA (one full-row gather stream; 528 B rows halve the
dominant HBM gather traffic vs a bf16 table), and reduces per segment
with one-hot matrices built ON-CHIP from per-edge srcrel indices
(iota/is_equal on DVE) - no dense one-hot DMA.  Per-edge s_src comes from
a per-block 4-col matmul against a host-pretransposed x shard (block node
ids are consecutive) expanded through the transposed one-hot on the PE.
fp8 h keeps rel-err ~8.6e-3 vs the fp32 reference (gate 2e-2), validated
by host-side simulation.
"""

import math
import os
import sys
from contextlib import ExitStack

import numpy as np

for _p in ("/opt/trn_rl_repo", "/root/.axon_site/_ro/trn_rl_repo"):
    if os.path.isdir(_p) and _p not in sys.path:
        sys.path.insert(0, _p)

import ml_dtypes

import concourse.bass as bass
import concourse.mybir as mybir
import concourse.tile as tile
from concourse import bacc
from concourse.bass_utils import run_bass_kernel_spmd
from concourse.masks import make_identity

# problem constants (hardcoded per contract)
N = 50000
F = 128
K = 4
NCORES = 8
NPC = N // NCORES          # 6250 nodes per core
BLK = 128                  # segment block = psum partition dim
NBLK = math.ceil(NPC / BLK)  # 49 blocks per core (last covers 106 nodes)
NPAD = NBLK * BLK          # 6272
M_SUB = 8                  # sub-tiles (128 edges each) per gather call
ROW = 16 + K * F           # 528 bytes: 4xf32 s_dst + 4x128 fp8 h
GRP = F + 1                # 129 accum cols per head: 128 msg + 1 denom
NCHUNK = math.ceil(N / 128)  # 391 node chunks in phase A
TROWS = NCHUNK * 128       # 50048 table rows (padded)
ALPHA = 0.01
EPS = 1e-8
PAD_SR = 500.0             # out-of-range srcrel => all-zero one-hot column

f32 = mybir.dt.float32
bf16 = mybir.dt.bfloat16
f8 = mybir.dt.float8e4
i32 = mybir.dt.int32

_CACHE = {}
_OPTS = {"at_mode": "pe"}   # "pe": AT via PE transpose; "dve": via iota/bcast


def _prep(x, e, weights, a, adj):
    """Host-side sharding/indexing prep. Only touches adj for structure
    plus dtype/layout conversions; all floating point math happens on
    device."""
    Et = adj.shape[1] + N
    src = np.concatenate([adj[0], np.arange(N, dtype=adj.dtype)]).astype(np.int64)
    dst = np.concatenate([adj[1], np.arange(N, dtype=adj.dtype)]).astype(np.int64)

    core = src // NPC
    rel = src - core * NPC
    blk = rel // BLK                    # < NBLK since rel < 6250
    srcrel = rel - blk * BLK
    key = (core * NBLK + blk).astype(np.int64)

    order = np.argsort(key, kind="stable")
    skey = key[order]
    ssrcrel = srcrel[order].astype(np.int64)
    sdst = dst[order]

    counts = np.bincount(skey, minlength=NCORES * NBLK)
    TPB = int(math.ceil(counts.max() / 128))          # sub-tiles per block
    S_TOT = NBLK * TPB
    G = math.ceil(S_TOT / M_SUB)
    S_PAD = G * M_SUB

    starts = np.zeros(NCORES * NBLK + 1, dtype=np.int64)
    np.cumsum(counts, out=starts[1:])
    rank = np.arange(Et, dtype=np.int64) - starts[skey]

    c_of = skey // NBLK
    b_of = skey % NBLK
    sub = b_of * TPB + rank // 128
    slot = rank % 128
    g_of = sub // M_SUB
    m_of = sub % M_SUB

    di_all = np.zeros((NCORES, G, 128, M_SUB), dtype=np.int32)
    sr_all = np.full((NCORES, G, 128, M_SUB), PAD_SR, dtype=ml_dtypes.bfloat16)
    srf_all = np.full((NCORES, G, 1, M_SUB * 128), PAD_SR,
                      dtype=ml_dtypes.bfloat16)
    di_all[c_of, g_of, slot, m_of] = sdst
    sr_all[c_of, g_of, slot, m_of] = ssrcrel
    srf_all[c_of, g_of, 0, m_of * 128 + slot] = ssrcrel

    xT = np.zeros((128, TROWS), dtype=ml_dtypes.bfloat16)
    xT[:, :N] = x.T
    xm = np.zeros((NCORES, NPAD, F), dtype=np.float32)
    em = np.zeros((NCORES, NPAD, K), dtype=np.float32)
    xmT = np.zeros((NCORES, 128, NPAD), dtype=ml_dtypes.bfloat16)
    for c in range(NCORES):
        xm[c, :NPC] = x[c * NPC:(c + 1) * NPC]
        em[c, :NPC] = e[c * NPC:(c + 1) * NPC]
        xmT[c, :, :NPC] = x[c * NPC:(c + 1) * NPC].T

    meta = dict(TPB=TPB, G=G, S_PAD=S_PAD)
    in_maps = []
    for c in range(NCORES):
        in_maps.append({
            "xT": xT,
            "w": np.ascontiguousarray(weights.astype(np.float32)),
            "a": np.ascontiguousarray(a.astype(np.float32)),
            "di": np.ascontiguousarray(di_all[c]),
            "sr": np.ascontiguousarray(sr_all[c]),
            "srf": np.ascontiguousarray(srf_all[c]),
            "xmT": np.ascontiguousarray(xmT[c]),
            "xm": np.ascontiguousarray(xm[c]),
            "em": np.ascontiguousarray(em[c]),
        })
    return meta, in_maps


def _build(meta, reps=1):
    """reps>1 replicates the whole kernel body (separate DRAM scratch per
    rep) purely as a timing instrument - differential wall time between
    rep counts isolates device time from the ~82 ms RPC floor."""
    TPB, G, S_PAD = meta["TPB"], meta["G"], meta["S_PAD"]
    nc = bacc.Bacc()

    xT = nc.declare_dram_parameter("xT", [128, TROWS], bf16, isOutput=False)
    W = nc.declare_dram_parameter("w", [K, 128, 128], f32, isOutput=False)
    Aa = nc.declare_dram_parameter("a", [K, 256, 1], f32, isOutput=False)
    DI = nc.declare_dram_parameter("di", [G, 128, M_SUB], i32, isOutput=False)
    SR = nc.declare_dram_parameter("sr", [G, 128, M_SUB], bf16, isOutput=False)
    SRF = nc.declare_dram_parameter("srf", [G, 1, M_SUB * 128], bf16,
                                    isOutput=False)
    XMT = nc.declare_dram_parameter("xmT", [128, NPAD], bf16, isOutput=False)
    XM = nc.declare_dram_parameter("xm", [NPAD, F], f32, isOutput=False)
    EM = nc.declare_dram_parameter("em", [NPAD, K], f32, isOutput=False)
    OUT = nc.declare_dram_parameter("out", [NPAD, F], f32, isOutput=True)

    Copy = mybir.ActivationFunctionType.Copy
    Exp = mybir.ActivationFunctionType.Exp
    Lrelu = mybir.ActivationFunctionType.Lrelu

    with ExitStack() as ctx:
        tc = ctx.enter_context(tile.TileContext(nc))
        const = ctx.enter_context(tc.tile_pool(name="const", bufs=1))
        sb = ctx.enter_context(tc.tile_pool(name="sb", bufs=3))
        sbG = ctx.enter_context(tc.tile_pool(name="sbG", bufs=3))
        dram = ctx.enter_context(tc.tile_pool(name="dram", bufs=1, space="DRAM"))

        ident = const.tile([128, 128], f32, tag="ident")
        make_identity(nc, ident[:])
        ident_bf = const.tile([128, 128], bf16, tag="ident_bf")
        nc.vector.tensor_copy(ident_bf[:], ident[:])
        # iota constants for on-chip one-hot builds (values <=127, exact bf16)
        # wide row iota: value = j % 128 across M_SUB*128 cols
        io_i = const.tile([128, M_SUB * 128], i32, tag="io_i")
        nc.gpsimd.iota(io_i[:], pattern=[[0, M_SUB], [1, 128]], base=0,
                       channel_multiplier=0)
        iota_row = const.tile([128, M_SUB * 128], bf16, tag="iota_row")
        nc.vector.tensor_copy(iota_row[:], io_i[:])
        if _OPTS["at_mode"] == "dve":
            pio_i = const.tile([128, 128], i32, tag="pio_i")
            nc.gpsimd.iota(pio_i[:], pattern=[[0, 128]], base=0,
                           channel_multiplier=1)
            iota_par = const.tile([128, 128], bf16, tag="iota_par")
            nc.vector.tensor_copy(iota_par[:], pio_i[:])

        # ---- one-time: wext[k] = [W_k | vb_k] bf16, vab = [va_0..va_3] ----
        wext = []
        vab = const.tile([128, K], bf16, tag="vab")
        with tc.tile_pool(name="psA1", bufs=1, space="PSUM") as psA1:
            for k in range(K):
                wkf = sb.tile([128, 128], f32, tag="wkf")
                nc.sync.dma_start(out=wkf[:], in_=W[k])
                ak = sb.tile([128, 2], f32, tag="ak")
                nc.sync.dma_start(out=ak[:, 0:1], in_=Aa[k, 0:128, :])
                nc.sync.dma_start(out=ak[:, 1:2], in_=Aa[k, 128:256, :])
                pT = psA1.tile([128, 128], f32, tag="pT")
                nc.tensor.transpose(pT[:], wkf[:], ident[:])
                wkT = sb.tile([128, 128], f32, tag="wkT")
                nc.vector.tensor_copy(wkT[:], pT[:])
                pva = psA1.tile([128, 2], f32, tag="pva")
                nc.tensor.matmul(pva[:], wkT[:], ak[:], start=True, stop=True)
                wk = const.tile([128, GRP], bf16, tag=f"wext{k}")
                nc.vector.tensor_copy(wk[:, 0:128], wkf[:])
                nc.vector.tensor_copy(wk[:, 128:129], pva[:, 1:2])
                nc.vector.tensor_copy(vab[:, k:k + 1], pva[:, 0:1])
                wext.append(wk)

        for rep in range(reps):
            Tmsg = dram.tile([TROWS, ROW], f8, tag=f"Tmsg{rep}")
            OUTt = OUT if rep == reps - 1 else \
                dram.tile([NPAD, F], f32, tag=f"outs{rep}")

            # ---- per-block s_src table: ssb[:, b*K:(b+1)*K] (bf16) ----
            ssb = const.tile([128, NBLK * K], bf16, tag=f"ssb{rep}")
            with tc.tile_pool(name="psS", bufs=2, space="PSUM") as psS:
                for b in range(NBLK):
                    xtb = sb.tile([128, 128], bf16, tag="xtb")
                    nc.sync.dma_start(out=xtb[:],
                                      in_=XMT[:, b * 128:(b + 1) * 128])
                    psb = psS.tile([128, K], f32, tag="psb")
                    nc.tensor.matmul(psb[:], xtb[:], vab[:], start=True,
                                     stop=True)
                    nc.vector.tensor_copy(ssb[:, b * K:(b + 1) * K], psb[:])

            # ---- phase A: node table Tmsg = [s_dst f32x4 | h8 4x128] ----
            with tc.tile_pool(name="psA", bufs=2, space="PSUM") as psA:
                for c in range(NCHUNK):
                    xc = sb.tile([128, 128], bf16, tag="xc")
                    nc.sync.dma_start(out=xc[:], in_=xT[:, c * 128:(c + 1) * 128])
                    p01 = psA.tile([128, 2 * GRP], f32, tag="p01")
                    p23 = psA.tile([128, 2 * GRP], f32, tag="p23")
                    for k in range(K):
                        pt = p01 if k < 2 else p23
                        j = (k % 2) * GRP
                        nc.tensor.matmul(pt[:, j:j + GRP], xc[:], wext[k][:],
                                         start=True, stop=True)
                    tsb = sb.tile([128, ROW], f8, tag="tsb")
                    p01v = p01[:].rearrange("p (k o) -> p k o", o=GRP)
                    p23v = p23[:].rearrange("p (k o) -> p k o", o=GRP)
                    h01 = tsb[:, 16:16 + 256].rearrange("p (k o) -> p k o", o=F)
                    h23 = tsb[:, 16 + 256:16 + 512].rearrange(
                        "p (k o) -> p k o", o=F)
                    nc.scalar.copy(out=h01, in_=p01v[:, :, 0:F])
                    nc.vector.tensor_copy(out=h23, in_=p23v[:, :, 0:F])
                    tsf = tsb[:, 0:16].bitcast(f32)   # [128, 4] s_dst
                    nc.vector.tensor_copy(out=tsf[:, 0:2], in_=p01v[:, :, F])
                    nc.scalar.copy(out=tsf[:, 2:4], in_=p23v[:, :, F])
                    nc.sync.dma_start(out=Tmsg[c * 128:(c + 1) * 128, :],
                                      in_=tsb[:])

            # ---- gather + segment reduction ----
            with tc.tile_pool(name="psHI", bufs=2, space="PSUM") as psHI, \
                    tc.tile_pool(name="psG", bufs=2, space="PSUM") as psG:
                curA = curB = None
                for g in range(G):
                    dib = sbG.tile([128, M_SUB], i32, tag="dib")
                    nc.sync.dma_start(out=dib[:], in_=DI[g])
                    srp = sbG.tile([128, M_SUB], bf16, tag="srp")
                    nc.sync.dma_start(out=srp[:], in_=SR[g])

                    # one-hot A tiles for the whole group in one DVE op
                    asb = sbG.tile([128, M_SUB * 128], bf16, tag="asb")
                    asbv = asb[:].rearrange("p (m j) -> p m j", j=128)
                    nc.vector.tensor_tensor(
                        out=asbv,
                        in0=iota_row[:].rearrange("p (m j) -> p m j", j=128),
                        in1=srp[:, :, None].to_broadcast([128, M_SUB, 128]),
                        op=mybir.AluOpType.is_equal)

                    graw = sbG.tile([128, M_SUB * ROW], f8, tag="graw")
                    for m in range(M_SUB):
                        # HW indirect DMA: one index per partition per call
                        nc.gpsimd.indirect_dma_start(
                            out=graw[:, m * ROW:(m + 1) * ROW], out_offset=None,
                            in_=Tmsg[:, :],
                            in_offset=bass.IndirectOffsetOnAxis(
                                ap=dib[:, m:m + 1], axis=0))

                    if _OPTS["at_mode"] == "dve":
                        srf = sbG.tile([1, M_SUB * 128], bf16, tag="srf")
                        nc.sync.dma_start(out=srf[:], in_=SRF[g])
                        srfb = sbG.tile([128, M_SUB * 128], bf16, tag="srfb")
                        nc.gpsimd.partition_broadcast(srfb[:], srf[:])

                    # per-edge s_src expansion through transposed one-hot
                    psU = psG.tile([128, M_SUB * K], f32, tag="psU")
                    for m in range(M_SUB):
                        s = g * M_SUB + m
                        b = min(s // TPB, NBLK - 1)
                        if _OPTS["at_mode"] == "dve":
                            atm = sbG.tile([128, 128], bf16, tag="atm")
                            nc.vector.tensor_tensor(
                                out=atm[:], in0=iota_par[:],
                                in1=srfb[:, m * 128:(m + 1) * 128],
                                op=mybir.AluOpType.is_equal)
                        else:
                            pT2 = psG.tile([128, 128], bf16, tag="pT2")
                            nc.tensor.transpose(
                                pT2[:], asb[:, m * 128:(m + 1) * 128],
                                ident_bf[:])
                            atm = sbG.tile([128, 128], bf16, tag="atm")
                            nc.scalar.copy(out=atm[:], in_=pT2[:])
                        nc.tensor.matmul(psU[:, m * K:(m + 1) * K], atm[:],
                                         ssb[:, b * K:(b + 1) * K],
                                         start=True, stop=True)

                    # u = s_src[src] + s_dst[dst]; w = exp(leakyrelu(u))
                    grawv = graw[:].rearrange("p (m r) -> p m r", r=ROW)
                    sdv = grawv[:, :, 0:16].bitcast(f32)   # [p, m, 4]
                    uv = sbG.tile([128, M_SUB * K], f32, tag="uv")
                    nc.vector.tensor_tensor(
                        out=uv[:].rearrange("p (m k) -> p m k", k=K),
                        in0=psU[:].rearrange("p (m k) -> p m k", k=K),
                        in1=sdv, op=mybir.AluOpType.add)
                    lr = sbG.tile([128, M_SUB * K], f32, tag="lr")
                    nc.scalar.activation(lr[:], uv[:], Lrelu, alpha=ALPHA)
                    wb = sbG.tile([128, M_SUB * K], bf16, tag="wb")
                    nc.scalar.activation(wb[:], lr[:], Exp)

                    # gs[:, m, k, 0:128] = w * h8 ; gs[:, m, k, 128] = w
                    gs = sbG.tile([128, M_SUB * K * GRP], bf16, tag="gs")
                    gsv = gs[:].rearrange("p (m k o) -> p m k o", k=K, o=GRP)
                    ghv = grawv[:, :, 16:16 + K * F].rearrange(
                        "p m (k o) -> p m k o", o=F)
                    wbv = wb[:].rearrange("p (m k) -> p m k", k=K)
                    nc.vector.tensor_tensor(
                        out=gsv[:, :, :, 0:F], in0=ghv,
                        in1=wbv[:, :, :, None].to_broadcast([128, M_SUB, K, F]),
                        op=mybir.AluOpType.mult)
                    nc.vector.tensor_copy(out=gsv[:, :, :, F], in_=wbv)

                    for m in range(M_SUB):
                        s = g * M_SUB + m
                        b = min(s // TPB, NBLK - 1)
                        first = (s == b * TPB)
                        if b < NBLK - 1:
                            last = (s == b * TPB + TPB - 1)
                        else:
                            last = (s == S_PAD - 1)
                        if first:
                            curA = psHI.tile([128, 2 * GRP], f32, tag="hiA")
                            curB = psHI.tile([128, 2 * GRP], f32, tag="hiB")
                        lhsT = asb[:, m * 128:(m + 1) * 128]
                        o = m * K * GRP
                        nc.tensor.matmul(curA[:], lhsT, gs[:, o:o + 2 * GRP],
                                         start=first, stop=last)
                        nc.tensor.matmul(curB[:], lhsT,
                                         gs[:, o + 2 * GRP:o + 4 * GRP],
                                         start=first, stop=last)
                        if last:
                            _finalize(nc, sb, b, curA, curB, XM, EM, OUTt, Copy)
    nc.finalize()
    return nc


def _finalize(nc, sb, b, hA, hB, XM, EM, OUT, Copy):
    r0, r1 = b * 128, (b + 1) * 128
    xb = sb.tile([128, F], f32, tag="xb")
    nc.sync.dma_start(out=xb[:], in_=XM[r0:r1, :])
    eb = sb.tile([128, K], f32, tag="eb")
    nc.sync.dma_start(out=eb[:], in_=EM[r0:r1, :])
    hAv = hA[:].rearrange("p (k o) -> p k o", o=GRP)
    hBv = hB[:].rearrange("p (k o) -> p k o", o=GRP)
    d4 = sb.tile([128, K], f32, tag="d4")
    nc.vector.tensor_scalar_add(d4[:, 0:2], hAv[:, :, F], EPS)
    nc.vector.tensor_scalar_add(d4[:, 2:4], hBv[:, :, F], EPS)
    r4 = sb.tile([128, K], f32, tag="r4")
    nc.vector.reciprocal(r4[:], d4[:])
    s4 = sb.tile([128, K], f32, tag="s4")
    nc.vector.tensor_mul(s4[:], r4[:], eb[:])
    t_ = [sb.tile([128, F], f32, tag=f"t{k}", name=f"t{k}") for k in range(K)]
    nc.vector.tensor_scalar_mul(t_[0][:], hAv[:, 0, 0:F], s4[:, 0:1])
    nc.scalar.activation(t_[1][:], hAv[:, 1, 0:F], Copy, scale=s4[:, 1:2])
    nc.vector.tensor_scalar_mul(t_[2][:], hBv[:, 0, 0:F], s4[:, 2:3])
    nc.scalar.activation(t_[3][:], hBv[:, 1, 0:F], Copy, scale=s4[:, 3:4])
    q0 = sb.tile([128, F], f32, tag="q0")
    nc.vector.tensor_add(q0[:], t_[0][:], t_[1][:])
    q1 = sb.tile([128, F], f32, tag="q1")
    nc.vector.tensor_add(q1[:], t_[2][:], t_[3][:])
    q2 = sb.tile([128, F], f32, tag="q2")
    nc.vector.tensor_add(q2[:], q0[:], q1[:])
    acc = sb.tile([128, F], f32, tag="acc")
    nc.vector.tensor_add(acc[:], q2[:], xb[:])
    nc.sync.dma_start(out=OUT[r0:r1, :], in_=acc[:])


def kernel(x, e, weights, a, adj):
    meta, in_maps = _prep(np.asarray(x), np.asarray(e), np.asarray(weights),
                          np.asarray(a), np.asarray(adj))
    ck = (meta["TPB"], meta["G"])
    if ck not in _CACHE:
        _CACHE[ck] = _build(meta)
    nc = _CACHE[ck]
    res = run_bass_kernel_spmd(nc, in_maps, list(range(NCORES)))
    out = np.empty((N, F), dtype=np.float32)
    for c in range(NCORES):
        out[c * NPC:(c + 1) * NPC] = res.results[c]["out"][:NPC]
    return out
